# revision 19
# baseline (speedup 1.0000x reference)
"""Bass/Tile TRN2 kernel for nn_MessageAggregation.

Computes: s = sum_n e2[n]; out = leaky_relu((e1+s) @ W1.T + (e1*s) @ W2.T)

Sharding: data-parallel over batch B=8192 across 8 NeuronCores (1024 rows
per core); W1/W2 replicated.

Per-core layout: SBUF [128 partitions, 1024 free]; partition p holds batch
rows 8p..8p+7 (4 KB contiguous per partition per DMA descriptor). The
kernel is DMA-bound (~32 MB of all_embeddings2 per core at ~405 GB/s).

The n-reduction runs entirely on DVE as ONE chained accumulate into a PSUM
tile (1.268 us per [128,1024] slice -- within 0.5% of the wire rate, so the
stream and the chain run in lockstep with no cross-engine coupling; earlier
variants that routed slices to GpSimd stalled the stream through
buffer-reuse semaphores on GpSimd's slow, late-starting queue). The final
add writes s directly to SBUF (f32r-rounded) for the tail transposes.

Output is computed TRANSPOSED (out_T[o, f]) with W stationary: 8 s-chunk
f32r transposes + 4 wide f32r single-pass matmuls. The e1 @ W1.T term is a
closed PSUM accumulation group issued at the head while PE is idle
(transposes must not interleave with an open accumulation group); the tail
reopens with start=False. Free position f = j*128 + p maps to batch row
8p + j; the host gather un-permutes with a reshape/transpose (not timed).
"""

import sys

for _p in ("/opt/trn_rl_repo",):
    if _p not in sys.path:
        sys.path.insert(0, _p)

import numpy as np

import concourse.bacc as bacc
import concourse.mybir as mybir
import concourse.tile as tile
from concourse.masks import make_identity
from concourse.bass_utils import run_bass_kernel_spmd

B, N, D = 8192, 64, 128
M = 8  # cores
BL = B // M  # 1024 rows per core
R = BL // 128  # chunks per core (8)
F = BL  # free width of the [128, F] working layout
H = F // 2
F32 = mybir.dt.float32
F32R = mybir.dt.float32r
NEG_SLOPE = 0.01
LRELU = mybir.ActivationFunctionType.Lrelu


def build(load_bufs: int = 9):
    nc = bacc.Bacc(
        "TRN2",
        target_bir_lowering=False,
        debug=False,
        enable_asserts=False,
        num_devices=M,
    )
    e1 = nc.dram_tensor("embedding1", [BL, D], F32, kind="ExternalInput").ap()
    e2 = nc.dram_tensor("all_embeddings2", [N, BL, D], F32, kind="ExternalInput").ap()
    w1 = nc.dram_tensor("W1", [D, D], F32, kind="ExternalInput").ap()
    w2 = nc.dram_tensor("W2", [D, D], F32, kind="ExternalInput").ap()
    out = nc.dram_tensor("out", [D, BL], F32, kind="ExternalOutput").ap()

    e1_r = e1.rearrange("(p r) d -> p (r d)", p=128)  # [128, 1024]
    e2_r = e2.rearrange("n (p r) d -> p n (r d)", p=128)  # [128, 64, 1024]

    # Small loads first (the DVE chain starts ~5us earlier), then 2MB
    # through the middle, tapered at the end so the final adds lag the
    # stream as little as possible. GpSimd takes 8 slices from loads 3-6
    # (completes by ~48us -- far off every critical path); DVE gets the
    # other 56.
    plan = (
        [(1, "D")] * 8
        + [(4, "DDDD")] * 7
        + [(4, "GDDD")]
        + [(4, "GGDD")] * 3
        + [(4, "GDDD")]
        + [(2, "DD")] * 2
        + [(1, "D")] * 4
    )
    assert sum(g for g, _ in plan) == N

    with tile.TileContext(nc) as tc:
        with (
            tc.tile_pool(name="const", bufs=1) as cpool,
            tc.tile_pool(name="load", bufs=load_bufs) as lpool,
            tc.tile_pool(name="act", bufs=1) as apool,
            tc.tile_pool(name="sdve", bufs=1, space="PSUM") as sdpool,
            tc.tile_pool(name="ops", bufs=1, space="PSUM") as opool,
            tc.tile_pool(name="trps", bufs=2, space="PSUM") as trpool,
        ):
            ident = cpool.tile([128, 128], F32)
            make_identity(nc, ident[:])
            ident_r = cpool.tile([128, 128], F32R)
            nc.scalar.copy(out=ident_r[:], in_=ident[:])

            # GpSimd warmup: force the tensor-op library load before the
            # stream (otherwise it reconfigures mid-stream, ~4us stall)
            gwarm = cpool.tile([128, 1], F32)
            nc.gpsimd.memset(gwarm[:], 0.0)
            nc.gpsimd.tensor_add(out=gwarm[:], in0=gwarm[:], in1=gwarm[:])

            w1_sb = cpool.tile([128, 128], F32)
            nc.scalar.dma_start(out=w1_sb[:], in_=w1)
            w2_sb = cpool.tile([128, 128], F32)
            nc.scalar.dma_start(out=w2_sb[:], in_=w2)
            e1_sb = apool.tile([128, F], F32)
            nc.scalar.dma_start(out=e1_sb[:], in_=e1_r)

            # ---- stream ----
            s_dve = sdpool.tile([128, F], F32)  # PSUM accumulator (2 banks)
            s_gps = apool.tile([128, F], F32)  # GpSimd partial
            s_sb = apool.tile([128, F], F32R)  # folded s (f32r) for the tail
            seen = {"D": 0, "G": 0}
            base = 0
            for li, (gl, routing) in enumerate(plan):
                t = lpool.tile([128, gl * F], F32, tag="load")
                # alternate the two HWDGE queues (sync/scalar): descriptor
                # generation, ring capacity and completion receipts overlap,
                # keeping the per-load cycle at the ~5.05us wire time (one
                # queue alone serializes at ~6us/load)
                dma_eng = nc.sync if li % 2 == 0 else nc.scalar
                dma_eng.dma_start(
                    out=t[:].rearrange("p (n f) -> p n f", n=gl),
                    in_=e2_r[:, base : base + gl, :],
                )
                for g in range(gl):
                    eng = routing[g]
                    seen[eng] += 1
                    sl = t[:, g * F : (g + 1) * F]
                    if eng == "D":
                        if seen["D"] == 1:
                            nc.vector.tensor_copy(out=s_dve[:], in_=sl)
                        else:
                            nc.vector.tensor_add(out=s_dve[:], in0=s_dve[:], in1=sl)
                    else:
                        if seen["G"] == 1:
                            nc.gpsimd.tensor_copy(out=s_gps[:], in_=sl)
                        else:
                            nc.gpsimd.tensor_add(out=s_gps[:], in0=s_gps[:], in1=sl)
                base += gl

            # W.T in SBUF: stationary operand of the output matmuls. fp32
            # for the exact e1-term at the head; f32r for the single-pass
            # tail matmuls.
            w1t_ps = trpool.tile([128, 128], F32, tag="tr")
            nc.tensor.transpose(w1t_ps[:], w1_sb[:], ident[:])
            w1t = cpool.tile([128, 128], F32)
            nc.scalar.copy(out=w1t[:], in_=w1t_ps[:])
            w1t_r = cpool.tile([128, 128], F32R)
            nc.scalar.copy(out=w1t_r[:], in_=w1t_ps[:])
            w2t_ps = trpool.tile([128, 128], F32, tag="tr")
            nc.tensor.transpose(w2t_ps[:], w2_sb[:], ident[:])
            w2t_r = cpool.tile([128, 128], F32R)
            nc.scalar.copy(out=w2t_r[:], in_=w2t_ps[:])

            # e1^T pre-stage: chunk j of e1 transposed -> e1t[:, j*128:(j+1)*128]
            e1t = apool.tile([128, F], F32)
            for j in range(R):
                sl = slice(j * 128, (j + 1) * 128)
                tp = trpool.tile([128, 128], F32, tag="tr")
                nc.tensor.transpose(tp[:], e1_sb[:, sl], ident[:])
                nc.scalar.copy(out=e1t[:, sl], in_=tp[:])

            # e1 @ W1.T term of out_T, as a CLOSED accumulation group per
            # half (PE idle during the stream; tail reopens with start=False).
            o_ps0 = opool.tile([128, H], F32)
            o_ps1 = opool.tile([128, H], F32)
            o_ps = [o_ps0, o_ps1]
            for h in range(2):
                hs = slice(h * H, (h + 1) * H)
                nc.tensor.matmul(
                    o_ps[h][:], lhsT=w1t[:], rhs=e1t[:, hs], start=True, stop=True
                )

            # ---- tail ----
            # Per chunk: f32r transpose of the s chunk, st = s^T (f32r matmul
            # rhs; cast split DVE/scalar), x2t = e1t * s^T. Per half: two
            # wide f32r matmuls accumulating onto the e1-term PSUM, lrelu,
            # store.
            st = apool.tile([128, F], F32R)
            x2t = apool.tile([128, F], F32R)
            out_sb = apool.tile([128, F], F32)
            for h in range(2):
                for jj in range(4):
                    j = h * 4 + jj
                    sl = slice(j * 128, (j + 1) * 128)
                    nc.vector.tensor_add(
                        out=s_sb[:, sl], in0=s_dve[:, sl], in1=s_gps[:, sl]
                    )
                    tp = trpool.tile([128, 128], F32R, tag="tr")
                    nc.tensor.transpose(tp[:], s_sb[:, sl], ident_r[:])
                    if jj % 2 == 0:
                        nc.scalar.copy(out=st[:, sl], in_=tp[:])
                    else:
                        nc.vector.tensor_copy(out=st[:, sl], in_=tp[:])
                    nc.vector.tensor_mul(out=x2t[:, sl], in0=e1t[:, sl], in1=tp[:])
                hs = slice(h * H, (h + 1) * H)
                nc.tensor.matmul(
                    o_ps[h][:], lhsT=w1t_r[:], rhs=st[:, hs], start=False, stop=False
                )
                nc.tensor.matmul(
                    o_ps[h][:], lhsT=w2t_r[:], rhs=x2t[:, hs], start=False, stop=True
                )
                nc.scalar.activation(
                    out_sb[:, hs], o_ps[h][:], LRELU, alpha=NEG_SLOPE
                )
                nc.scalar.dma_start(out=out[:, hs], in_=out_sb[:, hs])

    nc.compile()
    return nc


_NC = None


def _get_nc():
    global _NC
    if _NC is None:
        _NC = build()
    return _NC


def _make_in_maps(inputs):
    e1 = np.asarray(inputs["embedding1"], dtype=np.float32)
    e2 = np.asarray(inputs["all_embeddings2"], dtype=np.float32)
    w1 = np.asarray(inputs["W1"], dtype=np.float32)
    w2 = np.asarray(inputs["W2"], dtype=np.float32)
    in_maps = []
    for k in range(M):
        sl = slice(k * BL, (k + 1) * BL)
        in_maps.append(
            {
                "embedding1": np.ascontiguousarray(e1[sl]),
                "all_embeddings2": np.ascontiguousarray(e2[:, sl, :]),
                "W1": w1,
                "W2": w2,
            }
        )
    return in_maps


def _unshard(arr):
    # arr: out_T [o=128, f=1024] with f = j*128 + p <-> batch row 8p + j
    return arr.reshape(128, 8, 128).transpose(2, 1, 0).reshape(BL, D)


def _run(inputs, trace=False, **kwargs):
    nc = _get_nc()
    res = run_bass_kernel_spmd(
        nc, _make_in_maps(inputs), core_ids=list(range(M)), trace=trace, **kwargs
    )
    full = np.concatenate(
        [_unshard(res.results[k]["out"]) for k in range(M)], axis=0
    )
    return full, res


def kernel(**inputs):
    full, _ = _run(inputs)
    return full


# revision 20
# speedup vs baseline: 1.1323x; 1.1323x over previous
"""Bass/Tile TRN2 kernel for nn_MessageAggregation.

Computes: s = sum_n e2[n]; out = leaky_relu((e1+s) @ W1.T + (e1*s) @ W2.T)

Sharding: data-parallel over batch B=8192 across 8 NeuronCores (1024 rows
per core); W1/W2 replicated.

Per-core layout: SBUF [128 partitions, 1024 free]; partition p holds batch
rows 8p..8p+7 (4 KB contiguous per partition per DMA descriptor). The
kernel is DMA-bound (~32 MB of all_embeddings2 per core at ~405 GB/s).

The n-reduction is split DVE 43 / GpSimd 21 (1.27 vs ~2.5 us per
[128,1024] slice), alternating within each load so neither chain is ever
arrival-saturated. The partials meet only in the tail: each output chunk
folds s_dve + s_gps on DVE right before its transpose, so neither chain
ever blocks the other mid-stream (a mid-stream fold measurably stalled the
DVE queue ~7us behind GpSimd).

Output is computed TRANSPOSED (out_T[o, f]) with W stationary: 8 s-chunk
transposes + 4 wide f32r single-pass matmuls (f32r moving>=256 runs at
1 cycle/row vs 4 for fp32, scale-relative error ~1.4e-4 vs the 2e-2 gate).
The e1 @ W1.T term is a closed fp32 PSUM accumulation group issued at the
head while PE is idle (transposes must not interleave with an open
accumulation group); the tail reopens with start=False. Free position
f = j*128 + p maps to batch row 8p + j; the host gather un-permutes with a
reshape/transpose (not timed).
"""

import sys

for _p in ("/opt/trn_rl_repo",):
    if _p not in sys.path:
        sys.path.insert(0, _p)

import numpy as np

import concourse.bacc as bacc
import concourse.mybir as mybir
import concourse.tile as tile
from concourse.masks import make_identity
from concourse.bass_utils import run_bass_kernel_spmd

B, N, D = 8192, 64, 128
M = 8  # cores
BL = B // M  # 1024 rows per core
R = BL // 128  # chunks per core (8)
F = BL  # free width of the [128, F] working layout
H = F // 2
F32 = mybir.dt.float32
F32R = mybir.dt.float32r
NEG_SLOPE = 0.01
LRELU = mybir.ActivationFunctionType.Lrelu


def build(load_bufs: int = 6):
    nc = bacc.Bacc(
        "TRN2",
        target_bir_lowering=False,
        debug=False,
        enable_asserts=False,
        num_devices=M,
    )
    e1 = nc.dram_tensor("embedding1", [BL, D], F32, kind="ExternalInput").ap()
    e2 = nc.dram_tensor("all_embeddings2", [N, BL, D], F32, kind="ExternalInput").ap()
    w1 = nc.dram_tensor("W1", [D, D], F32, kind="ExternalInput").ap()
    w2 = nc.dram_tensor("W2", [D, D], F32, kind="ExternalInput").ap()
    out = nc.dram_tensor("out", [D, BL], F32, kind="ExternalOutput").ap()

    e1_r = e1.rearrange("(p r) d -> p (r d)", p=128)  # [128, 1024]
    e2_r = e2.rearrange("n (p r) d -> p n (r d)", p=128)  # [128, 64, 1024]

    # 14 x 2MB + 2 x 1MB + 4 x 512KB (tapered so the final adds lag the
    # stream as little as possible). D -> DVE, G -> GpSimd.
    plan = (
        [(4, "DDGD"), (4, "DGDG")] * 7
        + [(2, "DD")] * 2
        + [(1, "D")] * 4
    )
    assert sum(g for g, _ in plan) == N

    with tile.TileContext(nc) as tc:
        with (
            tc.tile_pool(name="const", bufs=1) as cpool,
            tc.tile_pool(name="load", bufs=load_bufs) as lpool,
            tc.tile_pool(name="act", bufs=1) as apool,
            tc.tile_pool(name="sdve", bufs=1, space="PSUM") as sdpool,
            tc.tile_pool(name="ops", bufs=1, space="PSUM") as opool,
            tc.tile_pool(name="trps", bufs=2, space="PSUM") as trpool,
        ):
            ident = cpool.tile([128, 128], F32)
            make_identity(nc, ident[:])

            w1_sb = cpool.tile([128, 128], F32)
            nc.scalar.dma_start(out=w1_sb[:], in_=w1)
            w2_sb = cpool.tile([128, 128], F32)
            nc.scalar.dma_start(out=w2_sb[:], in_=w2)
            e1_sb = apool.tile([128, F], F32)
            nc.scalar.dma_start(out=e1_sb[:], in_=e1_r)

            # W.T in SBUF: stationary operand of the output matmuls. fp32
            # for the exact e1-term at the head; f32r for the single-pass
            # tail matmuls.
            w1t_ps = trpool.tile([128, 128], F32, tag="tr")
            nc.tensor.transpose(w1t_ps[:], w1_sb[:], ident[:])
            w1t = cpool.tile([128, 128], F32)
            nc.scalar.copy(out=w1t[:], in_=w1t_ps[:])
            w1t_r = cpool.tile([128, 128], F32R)
            nc.scalar.copy(out=w1t_r[:], in_=w1t_ps[:])
            w2t_ps = trpool.tile([128, 128], F32, tag="tr")
            nc.tensor.transpose(w2t_ps[:], w2_sb[:], ident[:])
            w2t_r = cpool.tile([128, 128], F32R)
            nc.scalar.copy(out=w2t_r[:], in_=w2t_ps[:])

            # e1^T pre-stage: chunk j of e1 transposed -> e1t[:, j*128:(j+1)*128]
            e1t = apool.tile([128, F], F32)
            for j in range(R):
                sl = slice(j * 128, (j + 1) * 128)
                tp = trpool.tile([128, 128], F32, tag="tr")
                nc.tensor.transpose(tp[:], e1_sb[:, sl], ident[:])
                nc.scalar.copy(out=e1t[:, sl], in_=tp[:])

            # e1 @ W1.T term of out_T, as a CLOSED accumulation group per
            # half (PE idle during the stream; tail reopens with start=False).
            o_ps0 = opool.tile([128, H], F32)
            o_ps1 = opool.tile([128, H], F32)
            o_ps = [o_ps0, o_ps1]
            for h in range(2):
                hs = slice(h * H, (h + 1) * H)
                nc.tensor.matmul(
                    o_ps[h][:], lhsT=w1t[:], rhs=e1t[:, hs], start=True, stop=True
                )

            # ---- stream ----
            s_dve = sdpool.tile([128, F], F32)  # PSUM accumulator (2 banks)
            s_gps = apool.tile([128, F], F32)  # GpSimd SBUF accumulator
            seen = {"D": 0, "G": 0}
            base = 0
            for gl, routing in plan:
                t = lpool.tile([128, gl * F], F32, tag="load")
                nc.sync.dma_start(
                    out=t[:].rearrange("p (n f) -> p n f", n=gl),
                    in_=e2_r[:, base : base + gl, :],
                )
                for g in range(gl):
                    eng = routing[g]
                    seen[eng] += 1
                    sl = t[:, g * F : (g + 1) * F]
                    if eng == "D":
                        if seen["D"] == 1:
                            nc.vector.tensor_copy(out=s_dve[:], in_=sl)
                        else:
                            nc.vector.tensor_add(out=s_dve[:], in0=s_dve[:], in1=sl)
                    else:
                        if seen["G"] == 1:
                            nc.gpsimd.tensor_copy(out=s_gps[:], in_=sl)
                        else:
                            nc.gpsimd.tensor_add(out=s_gps[:], in0=s_gps[:], in1=sl)
                base += gl

            # ---- tail ----
            # Per chunk: fold the two partials (DVE), transpose, st = s^T
            # (f32r matmul rhs), x2t = e1t * s^T. Per half: two wide f32r
            # matmuls accumulating onto the e1-term PSUM, lrelu, store.
            s_sb = apool.tile([128, F], F32)
            st = apool.tile([128, F], F32R)
            x2t = apool.tile([128, F], F32R)
            out_sb = apool.tile([128, F], F32)
            for h in range(2):
                for jj in range(4):
                    j = h * 4 + jj
                    sl = slice(j * 128, (j + 1) * 128)
                    nc.vector.tensor_add(
                        out=s_sb[:, sl], in0=s_dve[:, sl], in1=s_gps[:, sl]
                    )
                    tp = trpool.tile([128, 128], F32, tag="tr")
                    nc.tensor.transpose(tp[:], s_sb[:, sl], ident[:])
                    nc.vector.tensor_copy(out=st[:, sl], in_=tp[:])
                    nc.vector.tensor_mul(out=x2t[:, sl], in0=e1t[:, sl], in1=tp[:])
                hs = slice(h * H, (h + 1) * H)
                nc.tensor.matmul(
                    o_ps[h][:], lhsT=w1t_r[:], rhs=st[:, hs], start=False, stop=False
                )
                nc.tensor.matmul(
                    o_ps[h][:], lhsT=w2t_r[:], rhs=x2t[:, hs], start=False, stop=True
                )
                nc.scalar.activation(
                    out_sb[:, hs], o_ps[h][:], LRELU, alpha=NEG_SLOPE
                )
                nc.scalar.dma_start(out=out[:, hs], in_=out_sb[:, hs])

    nc.compile()
    return nc


_NC = None


def _get_nc():
    global _NC
    if _NC is None:
        _NC = build()
    return _NC


def _make_in_maps(inputs):
    e1 = np.asarray(inputs["embedding1"], dtype=np.float32)
    e2 = np.asarray(inputs["all_embeddings2"], dtype=np.float32)
    w1 = np.asarray(inputs["W1"], dtype=np.float32)
    w2 = np.asarray(inputs["W2"], dtype=np.float32)
    in_maps = []
    for k in range(M):
        sl = slice(k * BL, (k + 1) * BL)
        in_maps.append(
            {
                "embedding1": np.ascontiguousarray(e1[sl]),
                "all_embeddings2": np.ascontiguousarray(e2[:, sl, :]),
                "W1": w1,
                "W2": w2,
            }
        )
    return in_maps


def _unshard(arr):
    # arr: out_T [o=128, f=1024] with f = j*128 + p <-> batch row 8p + j
    return arr.reshape(128, 8, 128).transpose(2, 1, 0).reshape(BL, D)


def _run(inputs, trace=False, **kwargs):
    nc = _get_nc()
    res = run_bass_kernel_spmd(
        nc, _make_in_maps(inputs), core_ids=list(range(M)), trace=trace, **kwargs
    )
    full = np.concatenate(
        [_unshard(res.results[k]["out"]) for k in range(M)], axis=0
    )
    return full, res


def kernel(**inputs):
    full, _ = _run(inputs)
    return full
